# revision 28
# baseline (speedup 1.0000x reference)
"""GroupDense kernel for Trainium2 (8 NeuronCores, SPMD data-parallel over batch).

y[b,s,g*64+v] = relu(sum_u x[b,s,g*64+u] * w[g,u,v])
x: [8, 2048, 4096] fp32, w: [64, 64, 64] fp32.

Per-core: core i processes batch i (2048 tokens). The host pre-transposes
x to xT [C, TOK] and packs the weights into 32 block-diagonal [128,128]
tiles (two 64x64 groups each), both cast to bf16. On-chip, each channel
block cb computes yT[cb] = relu(W_bd[cb].T @ xT[cb]) with the weights as
the PE stationary operand and 512-token moving strips, so no on-chip
transpose is needed and matmuls run at the bf16 rate. ReLU + bf16
downconvert run out of PSUM, split across the ACT and DVE engines. The
host transposes yT back and upcasts to fp32.
"""

import ml_dtypes
import numpy as np

import concourse.bass as bass
import concourse.mybir as mybir
import concourse.tile as tile
from concourse import bacc
from concourse.bass import ds, ts
from concourse.bass_utils import run_bass_kernel_spmd

B, S, C = 8, 2048, 4096
U = 64
G = C // U  # 64 groups
NCORES = 8
TOK = (B * S) // NCORES  # 2048 tokens per core
P = 128
CB = C // P   # 32 channel blocks (2 groups each)
NMOV = 512    # moving (token) strip per matmul = 1 PSUM bank of fp32
NSTRIP = TOK // NMOV  # 4

F32 = mybir.dt.float32
BF16 = mybir.dt.bfloat16
BF16_NP = ml_dtypes.bfloat16

_cached_nc = None


def _build():
    global _cached_nc
    if _cached_nc is not None:
        return _cached_nc

    nc = bacc.Bacc("TRN2", target_bir_lowering=False)

    xT_d = nc.dram_tensor("xT", [C, TOK], BF16, kind="ExternalInput")
    w_d = nc.dram_tensor("w", [P, CB * P], BF16, kind="ExternalInput")
    yT_d = nc.dram_tensor("yT", [C, TOK], BF16, kind="ExternalOutput")

    with tile.TileContext(nc) as tc:
        with (
            tc.tile_pool(name="wpool", bufs=1) as wpool,
            tc.tile_pool(name="xpool", bufs=10) as xpool,
            tc.tile_pool(name="ypool", bufs=5) as ypool,
            tc.tile_pool(name="ps", bufs=8, space="PSUM") as ps,
        ):
            # weights in 16 small chunks at the head of ACT's queue (the
            # first matmul only waits on 128KB), interleaved with pre-issued
            # x loads for cb 1,3,5,7 so BOTH HWDGE queues carry loads during
            # the stores-free ramp phase. Pre-issuing (before any relu enters
            # ACT's stream) keeps the issuance free of compute waits.
            WCH = 2  # channel blocks per weight chunk
            w_chunks = [
                wpool.tile([P, WCH, P], BF16, name=f"w_j{j}")
                for j in range(CB // WCH)
            ]

            def load_w(j):
                nc.scalar.dma_start(
                    w_chunks[j][:], w_d[:, ds(j * WCH * P, WCH * P)]
                )

            early_x = {}
            # x0 first: its quarter-loads head Sync's queue so the first
            # matmul's data is the first transfer on each queue; x1/x3 ride
            # the head of gpsimd's queue (its stores only start later)
            x0 = xpool.tile([P, TOK], BF16, name="x_cb")
            cw = TOK // 4
            load_w(0)
            for h in range(4):
                nc.sync.dma_start(
                    x0[:, ds(h * cw, cw)], xT_d[ds(0, P), ds(h * cw, cw)]
                )
            early_x[0] = x0
            load_w(1)
            for cb in (1, 3):
                x_cb = xpool.tile([P, TOK], BF16, name="x_cb")
                nc.gpsimd.dma_start(x_cb[:], xT_d[ts(cb, P), :])
                early_x[cb] = x_cb
            # only w0-w3 ride ACT's queue (2.6us of issuance before its
            # first relu, which gates the first store); the late-needed
            # chunks go to gpsimd, whose head is otherwise idle
            load_w(2)
            load_w(3)
            nc.gpsimd.dma_start(
                w_chunks[4][:], w_d[:, ds(4 * WCH * P, WCH * P)]
            )
            nc.gpsimd.dma_start(
                w_chunks[5][:], w_d[:, ds(5 * WCH * P, WCH * P)]
            )

            # Loads own Sync's queue mid-run (uncoupled from compute).
            # Stores get TWO mid-run queues so the pipeline isn't limited by
            # one ring's rate: even cb -> gpsimd immediately; odd cb -> ACT's
            # HWDGE lagged LAG iterations, so the relu-wait is already
            # satisfied when ACT reaches the issuance (no coupling stall).
            LAG = 2
            y_tiles = [None] * CB
            for cb in range(CB):
                if cb in early_x:
                    x_cb = early_x[cb]
                else:
                    x_cb = xpool.tile([P, TOK], BF16)
                    nc.sync.dma_start(x_cb[:], xT_d[ts(cb, P), :])
                if cb < 10:
                    # trickle w6..w15 through gpsimd between its stores;
                    # w_j is needed at iteration 2j, always >=2 ahead
                    j = cb + 6
                    nc.gpsimd.dma_start(
                        w_chunks[j][:], w_d[:, ds(j * WCH * P, WCH * P)]
                    )
                y_cb = ypool.tile([P, TOK], BF16)
                y_tiles[cb] = y_cb
                w_s = w_chunks[cb // WCH]
                for st in range(NSTRIP):
                    pY = ps.tile([P, NMOV], F32)
                    nc.tensor.matmul(
                        pY[:], w_s[:, cb % WCH, :], x_cb[:, ts(st, NMOV)],
                        start=True, stop=True,
                    )
                    if st % 2 == 0:
                        nc.scalar.activation(
                            y_cb[:, ts(st, NMOV)], pY[:],
                            mybir.ActivationFunctionType.Relu,
                        )
                    else:
                        nc.vector.tensor_scalar_max(
                            y_cb[:, ts(st, NMOV)], pY[:], 0.0
                        )
                if cb % 2 == 0 and cb < CB - 2:
                    nc.gpsimd.dma_start(yT_d[ts(cb, P), :], y_cb[:])
                elif cb % 2 == 1 and cb - LAG >= 1:
                    oc = cb - LAG
                    nc.scalar.dma_start(
                        yT_d[ts(oc, P), :], y_tiles[oc][:]
                    )
            # flush the last two tiles split across the two idle HWDGE
            # queues; gpsimd's SWDGE ring empties early so its end-of-kernel
            # drain stays off the critical path
            h = TOK // 2
            for cb in (CB - 2, CB - 1):
                nc.sync.dma_start(
                    yT_d[ts(cb, P), ds(0, h)], y_tiles[cb][:, ds(0, h)]
                )
                nc.scalar.dma_start(
                    yT_d[ts(cb, P), ds(h, h)], y_tiles[cb][:, ds(h, h)]
                )

    nc.compile()
    _cached_nc = nc
    return nc


def _pack_weights(kern):
    # block-diagonal pairs: [CB, P(u), P(v)] -> SBUF layout [P(u), CB, P(v)]
    w2 = np.zeros((CB, P, P), dtype=np.float32)
    w2[:, :U, :U] = kern[0::2]
    w2[:, U:, U:] = kern[1::2]
    w2 = w2.transpose(1, 0, 2).reshape(P, CB * P)
    return np.ascontiguousarray(w2.astype(BF16_NP))


def _prepare(x, kern):
    """Full inputs -> per-core in_maps (host shard + transpose + bf16 cast)."""
    x = np.asarray(x, dtype=np.float32)
    w = _pack_weights(np.asarray(kern, dtype=np.float32))
    return [
        {"xT": np.ascontiguousarray(x[i].T.astype(BF16_NP)), "w": w}
        for i in range(NCORES)
    ]


def _gather(results):
    """Per-core yT [C, TOK] bf16 -> full [B, S, C] fp32."""
    y = np.stack(
        [results[i]["yT"].astype(np.float32).T for i in range(NCORES)], axis=0
    )
    return np.ascontiguousarray(y.reshape(B, S, C))


def kernel(x, kernel):
    nc = _build()
    in_maps = _prepare(x, kernel)
    res = run_bass_kernel_spmd(nc, in_maps, list(range(NCORES)))
    return _gather(res.results)


# revision 29
# speedup vs baseline: 1.0627x; 1.0627x over previous
"""GroupDense kernel for Trainium2 (8 NeuronCores, SPMD data-parallel over batch).

y[b,s,g*64+v] = relu(sum_u x[b,s,g*64+u] * w[g,u,v])
x: [8, 2048, 4096] fp32, w: [64, 64, 64] fp32.

Per-core: core i processes batch i (2048 tokens). The host pre-transposes
x to xT [C, TOK] and packs the weights into 32 block-diagonal [128,128]
tiles (two 64x64 groups each), both cast to bf16. On-chip, each channel
block cb computes yT[cb] = relu(W_bd[cb].T @ xT[cb]) with the weights as
the PE stationary operand and 512-token moving strips, so no on-chip
transpose is needed and matmuls run at the bf16 rate. ReLU + bf16
downconvert run out of PSUM, split across the ACT and DVE engines. The
host transposes yT back and upcasts to fp32.
"""

import ml_dtypes
import numpy as np

import concourse.bass as bass
import concourse.mybir as mybir
import concourse.tile as tile
from concourse import bacc
from concourse.bass import ds, ts
from concourse.bass_utils import run_bass_kernel_spmd

B, S, C = 8, 2048, 4096
U = 64
G = C // U  # 64 groups
NCORES = 8
TOK = (B * S) // NCORES  # 2048 tokens per core
P = 128
CB = C // P   # 32 channel blocks (2 groups each)
NMOV = 512    # moving (token) strip per matmul = 1 PSUM bank of fp32
NSTRIP = TOK // NMOV  # 4

F32 = mybir.dt.float32
BF16 = mybir.dt.bfloat16
BF16_NP = ml_dtypes.bfloat16

_cached_nc = None


def _build():
    global _cached_nc
    if _cached_nc is not None:
        return _cached_nc

    nc = bacc.Bacc("TRN2", target_bir_lowering=False)

    xT_d = nc.dram_tensor("xT", [C, TOK], BF16, kind="ExternalInput")
    w_d = nc.dram_tensor("w", [P, CB * P], BF16, kind="ExternalInput")
    yT_d = nc.dram_tensor("yT", [C, TOK], BF16, kind="ExternalOutput")

    with tile.TileContext(nc) as tc:
        with (
            tc.tile_pool(name="wpool", bufs=1) as wpool,
            tc.tile_pool(name="xpool", bufs=10) as xpool,
            tc.tile_pool(name="ypool", bufs=5) as ypool,
            tc.tile_pool(name="ps", bufs=8, space="PSUM") as ps,
        ):
            # weights in 16 small chunks at the head of ACT's queue (the
            # first matmul only waits on 128KB), interleaved with pre-issued
            # x loads for cb 1,3,5,7 so BOTH HWDGE queues carry loads during
            # the stores-free ramp phase. Pre-issuing (before any relu enters
            # ACT's stream) keeps the issuance free of compute waits.
            WCH = 4  # channel blocks per weight chunk
            w_chunks = [
                wpool.tile([P, WCH, P], BF16, name=f"w_j{j}")
                for j in range(CB // WCH)
            ]

            def load_w(j):
                nc.scalar.dma_start(
                    w_chunks[j][:], w_d[:, ds(j * WCH * P, WCH * P)]
                )

            early_x = {}
            # x0 first: its quarter-loads head Sync's queue so the first
            # matmul's data is the first transfer on each queue; x1/x3 ride
            # the head of gpsimd's queue (its stores only start later)
            x0 = xpool.tile([P, TOK], BF16, name="x_cb")
            cw = TOK // 4
            load_w(0)
            for h in range(4):
                nc.sync.dma_start(
                    x0[:, ds(h * cw, cw)], xT_d[ds(0, P), ds(h * cw, cw)]
                )
            early_x[0] = x0
            load_w(1)
            for cb in (1, 3):
                x_cb = xpool.tile([P, TOK], BF16, name="x_cb")
                nc.gpsimd.dma_start(x_cb[:], xT_d[ts(cb, P), :])
                early_x[cb] = x_cb
            for j in range(2, CB // WCH):
                load_w(j)

            # Loads own Sync's queue mid-run (uncoupled from compute).
            # Stores get TWO mid-run queues so the pipeline isn't limited by
            # one ring's rate: even cb -> gpsimd immediately; odd cb -> ACT's
            # HWDGE lagged LAG iterations, so the relu-wait is already
            # satisfied when ACT reaches the issuance (no coupling stall).
            LAG = 2
            y_tiles = [None] * CB
            for cb in range(CB):
                if cb in early_x:
                    x_cb = early_x[cb]
                else:
                    x_cb = xpool.tile([P, TOK], BF16)
                    nc.sync.dma_start(x_cb[:], xT_d[ts(cb, P), :])
                y_cb = ypool.tile([P, TOK], BF16)
                y_tiles[cb] = y_cb
                w_s = w_chunks[cb // WCH]
                for st in range(NSTRIP):
                    pY = ps.tile([P, NMOV], F32)
                    nc.tensor.matmul(
                        pY[:], w_s[:, cb % WCH, :], x_cb[:, ts(st, NMOV)],
                        start=True, stop=True,
                    )
                    if st % 2 == 0:
                        nc.scalar.activation(
                            y_cb[:, ts(st, NMOV)], pY[:],
                            mybir.ActivationFunctionType.Relu,
                        )
                    else:
                        nc.vector.tensor_scalar_max(
                            y_cb[:, ts(st, NMOV)], pY[:], 0.0
                        )
                if cb % 2 == 0:
                    nc.gpsimd.dma_start(yT_d[ts(cb, P), :], y_cb[:])
                elif cb % 2 == 1 and cb - LAG >= 1:
                    oc = cb - LAG
                    nc.scalar.dma_start(
                        yT_d[ts(oc, P), :], y_tiles[oc][:]
                    )
            # flush the last odd store, split across the two idle HWDGE
            # queues so the final drain uses both
            h = TOK // 2
            nc.sync.dma_start(
                yT_d[ts(CB - 1, P), ds(0, h)], y_tiles[CB - 1][:, ds(0, h)]
            )
            nc.scalar.dma_start(
                yT_d[ts(CB - 1, P), ds(h, h)], y_tiles[CB - 1][:, ds(h, h)]
            )

    nc.compile()
    _cached_nc = nc
    return nc


def _pack_weights(kern):
    # block-diagonal pairs: [CB, P(u), P(v)] -> SBUF layout [P(u), CB, P(v)]
    w2 = np.zeros((CB, P, P), dtype=np.float32)
    w2[:, :U, :U] = kern[0::2]
    w2[:, U:, U:] = kern[1::2]
    w2 = w2.transpose(1, 0, 2).reshape(P, CB * P)
    return np.ascontiguousarray(w2.astype(BF16_NP))


def _prepare(x, kern):
    """Full inputs -> per-core in_maps (host shard + transpose + bf16 cast)."""
    x = np.asarray(x, dtype=np.float32)
    w = _pack_weights(np.asarray(kern, dtype=np.float32))
    return [
        {"xT": np.ascontiguousarray(x[i].T.astype(BF16_NP)), "w": w}
        for i in range(NCORES)
    ]


def _gather(results):
    """Per-core yT [C, TOK] bf16 -> full [B, S, C] fp32."""
    y = np.stack(
        [results[i]["yT"].astype(np.float32).T for i in range(NCORES)], axis=0
    )
    return np.ascontiguousarray(y.reshape(B, S, C))


def kernel(x, kernel):
    nc = _build()
    in_maps = _prepare(x, kernel)
    res = run_bass_kernel_spmd(nc, in_maps, list(range(NCORES)))
    return _gather(res.results)
